# revision 1
# baseline (speedup 1.0000x reference)
"""Box-from-mask kernel for Trainium2 (8 NeuronCores, SPMD data-parallel).

Problem: masks [100, 800, 1280] f32 -> boxes [100, 2, 2] f32 where
box[n] = [[xmin, ymin], [xmax, ymax]] of {(y, x) : masks[n, y, x] > 0.5},
with empty-mask sentinels xmin=W, ymin=H, xmax=-1, ymax=-1.

Flat-row sharding: the 100*800 = 80,000 mask rows are treated as one flat
[80000, 1280] array. Core c owns rows [c*10000, (c+1)*10000): 80 uniform
125-row tiles per core, no runt DMAs and zero duplicate traffic (125-row
tiles divide the row count exactly; engine time is set by the 1280-wide
free axis, so the 3 idle partitions are free).

Per-core device pipeline, per [125, 1280] block:
  - one DVE tensor_scalar(is_gt 0.5) -> 0/1 fp8 block, with accum_out(max)
    giving the per-row "any pixel" bit (one elementwise pass per element).
  - PE selector matmul (fp8 x fp8 -> fp32 PSUM, exact for 0/1 counts)
    accumulates per-column counts into three per-chunk [13, 512] PSUM
    tiles; the [125, 13] one-hot selector for each block routes every SBUF
    partition (= one mask row) to its mask's PSUM row, so blocks that
    straddle a mask boundary need no special casing.
Blocks arrive in partition-major DMA groups (9 tiles -> one 5.8 MB DMA,
each partition reading 46 KB contiguous) alternating between the SP and
ACT HWDGE rings; the final groups taper (4+2+1+1 tiles, all on SP so they
complete in FIFO order). The last TWO blocks skip the PE and ship their
binarized fp8 tiles raw (160 KB each; rows AND columns fold host-side),
so the PSUM chains stop at a block that lands ~3.6 us before the last
byte and the whole counts convert+flush (one wide Identity activation
draining PSUM on ACT + one DMA) hides under the final transfers.
blast78's write rides SP, whose FIFO delays it until the input stream has
drained. The very last tile is fetched as two half-column DMAs into
independent tiles, so the left half binarizes and ships while the right
half is still in flight - the serial work after the very last byte is one
0.4 us half-binarize plus one 80 KB output DMA round-trip. Outputs are
row-any bits [125, 78], raw fp16 column counts [13, 1280], and the two
fp8 tiles; the min/max index arithmetic happens host-side (exact).
"""

import sys

for _p in ("/opt/trn_rl_repo", "/opt/pypackages"):
    if _p not in sys.path:
        sys.path.append(_p)

import ml_dtypes
import numpy as np

import concourse.tile as tile
from concourse import bacc, mybir
from concourse.bass_utils import run_bass_kernel_spmd

N, H, W = 100, 800, 1280
N_CORES = 8
THRESHOLD = 0.5

ROWS = N * H  # 80,000 flat rows
P = 125  # rows per tile: 80,000 = 8 cores * 80 tiles * 125 rows, exactly.
# 128-row tiles would need 632 tiles for a uniform 8-way split (625 real),
# i.e. 7 tiles of duplicate traffic; 125-row tiles split evenly with zero
# overlap, and engine time is set by the 1280-wide free axis, not the
# partition count, so the 3 idle partitions cost nothing.
SHARD_ROWS = ROWS // N_CORES  # 10,000
NB = SHARD_ROWS // P  # 80 blocks of 125 rows
GSZ = 13  # max distinct masks touched by one core's shard
# DMA groups: sizes of consecutive tile groups fetched by one DMA each.
# Tapered tail: the short final groups shrink the serial compute left
# after the last transfer lands.
GROUP_SIZES = [9] * 8 + [4, 2, 1, 1]
assert sum(GROUP_SIZES) == NB

fp32 = mybir.dt.float32
fp16 = mybir.dt.float16
bf16 = mybir.dt.bfloat16
fp8 = mybir.dt.float8e4
Op = mybir.AluOpType


def _chunks(w):
    return [(c, min(512, w - c)) for c in range(0, w, 512)]


def _groups():
    """[(row_offset, n_tiles)] per DMA group."""
    out, r = [], 0
    for t in GROUP_SIZES:
        out.append((r, t))
        r += t * P
    return out


def _local_rows():
    """local_rows[p, B] = shard-local row held by partition p for block B.

    Partition-major DMA: group (R, T) lands rows R + p*T + a on partition p,
    column-block a.
    """
    cols = []
    for R, T in _groups():
        for a in range(T):
            cols.append(R + np.arange(P) * T + a)
    return np.stack(cols, axis=1)  # [P, NB]


LOCAL_ROWS = _local_rows()

RAW_BUFS = 4
BIN_BUFS = 6


def build_program():
    """One-core Bass/Tile program; run SPMD on all 8 cores."""
    chunks = _chunks(W)
    groups = _groups()
    tmax = max(t for _, t in groups)

    nc = bacc.Bacc(
        "TRN2", target_bir_lowering=False, debug=False, enable_asserts=False
    )
    masks = nc.dram_tensor("masks", [SHARD_ROWS, W], fp32, kind="ExternalInput").ap()
    # fp8 halves the selector's HBM traffic and the binarized tiles' SBUF
    # footprint; 0/1 are exact in fp8e4 and the fp8xfp8 matmul accumulates
    # in fp32 PSUM, so everything stays exact
    sel = nc.dram_tensor("sel", [128, NB * GSZ], fp8, kind="ExternalInput").ap()
    # NB-2 columns: only blocks 0..NB-3 accumulate row-any here. The last
    # two blocks' row-any comes from their shipped blast tiles host-side,
    # so this tile's last writer is block NB-3 and its DMA issues before
    # the final transfers even land.
    rowany_out = nc.dram_tensor(
        "rowany_out", [128, NB - 2], fp32, kind="ExternalOutput"
    ).ap()
    counts_out = nc.dram_tensor(
        "counts_out", [GSZ, W], fp16, kind="ExternalOutput"
    ).ap()
    # the last two blocks' binarized tiles, shipped raw: their reductions
    # happen host-side, so the PSUM chains stop at block NB-3 (which lands
    # ~3.6us before the last byte) and the whole counts convert+flush hides
    # under the final transfers instead of trailing them
    blast78_out = nc.dram_tensor(
        "blast78_out", [P, W], fp8, kind="ExternalOutput"
    ).ap()
    blast_out = nc.dram_tensor("blast_out", [P, W], fp8, kind="ExternalOutput").ap()

    with tile.TileContext(nc) as tc:
        with (
            tc.tile_pool(name="raw", bufs=RAW_BUFS) as rawp,
            tc.tile_pool(name="bin", bufs=BIN_BUFS) as binp,
            tc.tile_pool(name="consts", bufs=1) as constp,
            tc.tile_pool(name="psum", bufs=1, space="PSUM") as psump,
        ):
            # selector rides the gpsimd SWDGE queue so the SP/ACT HWDGE
            # queues start streaming mask tiles immediately
            sel_t = constp.tile([128, NB * GSZ], fp8)
            nc.gpsimd.dma_start(sel_t[:], sel)
            rowany = constp.tile([128, NB - 2], fp32)
            nc.gpsimd.memset(rowany[:], 0.0)
            csb = constp.tile([GSZ, W], fp16)
            # one PSUM tile: all three chunk chains stop together at block
            # NB-3 now, so a single wide csb beats three per-chunk converts
            cc = psump.tile([GSZ, W], fp32, name="cc", tag="cc")

            b_idx = 0
            n_taper = sum(1 for _, t in groups if t < max(GROUP_SIZES))
            for gi, (R, T) in enumerate(groups):
                if gi == len(groups) - 2:
                    # Block NB-2 bypasses the PE like the final block: one
                    # full-width binarize, tile shipped raw on SP, whose
                    # FIFO delays the HBM write until the input stream has
                    # drained (no read/write bus contention).
                    assert T == 1
                    bl78 = constp.tile([P, W], fp8, name="blast78")
                    raw78 = rawp.tile([128, tmax * W], fp32, tag="raw")
                    nc.sync.dma_start(raw78[:P, :W], masks[R : R + P, :])
                    nc.vector.tensor_scalar(
                        out=bl78[:, :],
                        in0=raw78[:P, :W],
                        scalar1=THRESHOLD,
                        scalar2=None,
                        op0=Op.is_gt,
                    )
                    nc.sync.dma_start(blast78_out, bl78[:, :])
                    b_idx += 1
                    continue
                if gi == len(groups) - 1:
                    # Final tile arrives as two half-column fetches into
                    # independent tiles: the left half binarizes and ships
                    # while the right half is still in flight, halving the
                    # serial work left after the very last byte lands.
                    assert T == 1
                    for hf, (c0, cw) in enumerate(((0, W // 2), (W // 2, W // 2))):
                        # dedicated small tile: the final fetches must never
                        # wait on raw-pool slot recycling (which is paced by
                        # binarize progress)
                        rawh = constp.tile([P, W // 2], fp32, name=f"rawh{hf}")
                        nc.sync.dma_start(
                            rawh[:P, :cw], masks[R : R + P, c0 : c0 + cw]
                        )
                        bl = constp.tile([P, W // 2], fp8, name=f"blast{hf}")
                        # no accum_out: this block's row-any is derived
                        # host-side from the blast tile itself
                        nc.vector.tensor_scalar(
                            out=bl[:, :],
                            in0=rawh[:P, :cw],
                            scalar1=THRESHOLD,
                            scalar2=None,
                            op0=Op.is_gt,
                        )
                        # SP ring: ACT's end-game is busy with the counts path
                        nc.sync.dma_start(blast_out[:, c0 : c0 + cw], bl[:, :])
                    b_idx += 1
                    continue
                raw = rawp.tile([128, tmax * W], fp32, tag="raw")
                # Bulk groups alternate the two HWDGE rings (SP/ACT) so
                # descriptor generation always overlaps a drain. The tapered
                # tail groups all ride SP: FIFO order within one ring makes
                # them complete in order, so the trailing compute pipelines
                # with the remaining transfers instead of stacking up after
                # a simultaneous round-robin finish.
                if gi >= len(groups) - n_taper:
                    eng = nc.sync
                else:
                    eng = nc.sync if gi % 2 == 0 else nc.scalar
                eng.dma_start(
                    raw[:P, : T * W],
                    masks[R : R + P * T, :].rearrange("(p a) x -> p (a x)", p=P),
                )
                if gi == 1:
                    # warmup AFTER ACT's first input DMA in program order:
                    # pulls the one-time LoadActFuncSet (~1.3us) into idle
                    # time without delaying the ACT ring's first transfer
                    # (it would otherwise land in the end-game csb)
                    nc.scalar.activation(
                        out=csb[:1, :1],
                        in_=rowany[:1, :1],
                        func=mybir.ActivationFunctionType.Identity,
                    )
                for a in range(T):
                    b = binp.tile([128, W], fp8, tag="b")
                    nc.vector.tensor_scalar(
                        out=b[:P, :],
                        in0=raw[:P, a * W : (a + 1) * W],
                        scalar1=THRESHOLD,
                        scalar2=None,
                        op0=Op.is_gt,
                        op1=Op.max,
                        accum_out=rowany[:P, b_idx : b_idx + 1],
                    )
                    for c0, cw in chunks:
                        nc.tensor.matmul(
                            cc[:, c0 : c0 + cw],
                            sel_t[:P, b_idx * GSZ : b_idx * GSZ + GSZ],
                            b[:P, c0 : c0 + cw],
                            start=(b_idx == 0),
                            stop=(b_idx == NB - 3),
                        )
                    b_idx += 1

            # Raw fp16 counts (exact: integers <= 800), one wide Identity
            # activation draining the whole PSUM region on ACT - the
            # canonical pattern, and entirely off DVE whose only end-game
            # work is the bypass binarizes. The host's presence check
            # (counts > 0) is unchanged. One merged counts DMA right behind
            # it on ACT's own ring.
            # (gpsimd stt from PSUM passes CoreSim but fails neuronx compile.)
            nc.scalar.activation(
                out=csb[:],
                in_=cc[:, :],
                func=mybir.ActivationFunctionType.Identity,
            )
            nc.scalar.dma_start(counts_out, csb[:])
            # gpsimd SWDGE: its ~1us generation runs parallel to the SP/ACT
            # end-game and finishes before the counts path
            nc.gpsimd.dma_start(rowany_out, rowany[:])

    nc.compile()
    return nc


def make_sel(core):
    """Per-block one-hot selector: partition p -> local mask index."""
    g = core * SHARD_ROWS + LOCAL_ROWS  # [P, NB] global rows
    first = (core * SHARD_ROWS) // H
    ul = g // H - first
    assert ul.min() >= 0 and ul.max() < GSZ
    sel = np.zeros((128, NB * GSZ), ml_dtypes.float8_e4m3)
    sel[np.arange(P)[:, None], np.arange(NB)[None, :] * GSZ + ul] = 1
    return sel


def postprocess(results):
    """Per-core rowany/counts -> boxes [N, 2, 2] f32 (exact)."""
    v1 = np.zeros(N)  # H - ymin   (0 if empty)
    v2 = np.zeros(N)  # ymax + 1
    u1 = np.zeros(N)  # W - xmin
    u2 = np.zeros(N)  # xmax + 1
    xs = np.arange(W)
    for c, r in enumerate(results):
        g = c * SHARD_ROWS + LOCAL_ROWS
        unit = g // H
        y = g % H
        a = np.asarray(r["rowany_out"])[:P] > 0  # [P, NB-2], blocks 0..NB-3
        ub, yb = unit[:, : NB - 2], y[:, : NB - 2]
        np.maximum.at(v1, ub[a], (H - yb)[a])
        np.maximum.at(v2, ub[a], (yb + 1)[a])
        first = (c * SHARD_ROWS) // H
        nu = ((c + 1) * SHARD_ROWS - 1) // H - first + 1
        p = np.asarray(r["counts_out"][:nu]) > 0  # [nu, W]
        np.maximum.at(u1, first + np.arange(nu), np.max(np.where(p, W - xs, 0), 1))
        np.maximum.at(u2, first + np.arange(nu), np.max(np.where(p, xs + 1, 0), 1))
        # the last two blocks bypassed the PE and skipped rowany: fold both
        # rows and columns in from their shipped binarized tiles
        for name, bcol in (("blast78_out", NB - 2), ("blast_out", NB - 1)):
            blast = np.asarray(r[name]) > 0  # [P, W]
            ublk = unit[:, bcol]  # [P] global mask id per partition
            yblk = y[:, bcol]
            pr = blast.any(1)
            np.maximum.at(v1, ublk[pr], (H - yblk)[pr])
            np.maximum.at(v2, ublk[pr], (yblk + 1)[pr])
            for uu in np.unique(ublk):
                colany = blast[ublk == uu].any(0)
                u1[uu] = max(u1[uu], np.where(colany, W - xs, 0).max())
                u2[uu] = max(u2[uu], np.where(colany, xs + 1, 0).max())
    boxes = np.empty((N, 2, 2), np.float32)
    boxes[:, 0, 0] = W - u1  # xmin
    boxes[:, 0, 1] = H - v1  # ymin
    boxes[:, 1, 0] = u2 - 1  # xmax
    boxes[:, 1, 1] = v2 - 1  # ymax
    return boxes


_cache = {}


def _get_program():
    if "nc" not in _cache:
        _cache["nc"] = build_program()
        _cache["sel"] = [make_sel(c) for c in range(N_CORES)]
    return _cache["nc"], _cache["sel"]


def make_in_maps(masks):
    masks = np.ascontiguousarray(np.asarray(masks, dtype=np.float32))
    _, sels = _get_program()
    flat = masks.reshape(ROWS, W)
    return [
        {"masks": flat[c * SHARD_ROWS : (c + 1) * SHARD_ROWS], "sel": sels[c]}
        for c in range(N_CORES)
    ]


def kernel(masks):
    nc, _ = _get_program()
    in_maps = make_in_maps(masks)
    res = run_bass_kernel_spmd(nc, in_maps, core_ids=list(range(N_CORES)))
    return postprocess(res.results)



# revision 66
# speedup vs baseline: 1.6792x; 1.6792x over previous
"""Box-from-mask kernel for Trainium2 (8 NeuronCores, SPMD data-parallel).

Problem: masks [100, 800, 1280] f32 -> boxes [100, 2, 2] f32 where
box[n] = [[xmin, ymin], [xmax, ymax]] of {(y, x) : masks[n, y, x] > 0.5}.

The 100*800 = 80,000 mask rows are one flat [80000, 1280] array; core c owns
rows [c*10000, (c+1)*10000).

Per-core pipeline, FOUR parallel DMA lanes (fully concurrent in the model):
  - two engine lanes: plain HWDGE dma_start on SP / ACT; the transfer holds
    the issuing engine (~1974ns per 125-row tile), disjoint row ranges.
  - one descriptor lane: prepared SWDGE dma_gathers (row-index gathers,
    128-row tiles) fired by trigger_dma. Only the ~1067ns/tile descriptor
    GENERATION holds the Pool engine; the transfers ride the shared DMA
    engines and overlap everything. Q7 reads the idx table per core, so the
    int16 idxs are replicated across all 128 partitions; the final gather
    pads with duplicate row indices (dupes only inflate counts, and the
    decode only tests count > 0, so they are harmless). Consumers
    synchronize on the DMA-completion semaphore via wait_ge.
Binarize (the 4th resource, overlapped): DVE tensor_scalar(is_gt 0.5) ->
0/1 fp8 with accum_out(max); ~15 tiles instead use ACT Relu(x - 0.5) ->
fp16 with accum_out(sum). Relu keeps exactness: any f32 delta above the
threshold is >= 2^-24, which fp16 cannot round to zero, and sums of
non-negatives stay positive, so "any hit" == "value > 0" for both kinds.
Column reduction: FLIPPED PE matmuls - binary chunk [rows,128] STATIONARY,
per-tile one-hot selector [rows,13] MOVING, accumulating [128 cols,
13 masks] per chunk into one PSUM region per lane (~10ns per matmul, PE
ingests everything for ~8us). Each lane's region stops at its last tile,
drains via DVE copy -> fp16 -> DMA out on the lane's own queue, hiding
under the other lanes' remaining input traffic.
Host: tiny exact "> 0" decode of rowany/counts -> boxes.
"""

import sys

for _p in ("/opt/trn_rl_repo", "/opt/pypackages"):
    if _p not in sys.path:
        sys.path.append(_p)

import ml_dtypes
import numpy as np

import concourse.tile as tile
from concourse import bacc, mybir
from concourse.bass_utils import run_bass_kernel_spmd

N, H, W = 100, 800, 1280
N_CORES = 8
THRESHOLD = 0.5

ROWS = N * H
SHARD_ROWS = ROWS // N_CORES  # 10,000
GSZ = 13  # max distinct masks touched by one core's shard
NCHUNK = W // 128

# Lane layout: two plain engine lanes (125-row tiles) and one gather lane
# (128-row tiles). Group sizes ramp up so compute starts early.
SP_GROUPS = [1, 2, 4, 4, 4, 4, 4, 2]
ACT_GROUPS = [1, 2, 3, 3, 3]
N_SP, G_SP = sum(SP_GROUPS), max(SP_GROUPS)
N_ACT, G_ACT = sum(ACT_GROUPS), max(ACT_GROUPS)
PR = 125
N_PLAIN = N_SP + N_ACT
GT_ROWS0 = PR * N_PLAIN
N_GT = -(-(SHARD_ROWS - GT_ROWS0) // 128)
K_GT = 4
GT_GROUPS = [2, 3] + [K_GT] * ((N_GT - 5) // K_GT) + (
    [(N_GT - 5) % K_GT] if (N_GT - 5) % K_GT else [])
assert sum(GT_GROUPS) == N_GT and max(GT_GROUPS) <= K_GT
NTILES = N_PLAIN + N_GT
N_RELU = 16  # tiles binarized on ACT (Relu) instead of DVE

fp32 = mybir.dt.float32
fp16 = mybir.dt.float16
fp8 = mybir.dt.float8e4
i16 = mybir.dt.int16
Op = mybir.AluOpType
AF = mybir.ActivationFunctionType

LANES = (
    dict(name="sp", kind="plain", n=N_SP, g=G_SP, eng="sync", gs=SP_GROUPS),
    dict(name="act", kind="plain", n=N_ACT, g=G_ACT, eng="scalar",
         gs=ACT_GROUPS),
    dict(name="gt", kind="gather", n=N_GT, g=K_GT, eng=None, gs=GT_GROUPS),
)


def _build_tables():
    """Static per-tile tables: lane, rows-per-partition map, engine."""
    lane_of = np.zeros(NTILES, np.int64)
    idx_in = np.zeros(NTILES, np.int64)
    nrows = np.zeros(NTILES, np.int64)
    local_rows = np.zeros((128, NTILES), np.int64)
    groups = []
    t0 = 0
    r0 = 0
    for li, L in enumerate(LANES):
        gl = []
        t = 0
        for take in L["gs"]:
            gl.append((t0 + t, take))
            if L["kind"] == "plain":
                R = r0 + t * PR
                for a in range(take):
                    tt = t0 + t + a
                    local_rows[:PR, tt] = R + np.arange(PR) * take + a
                    local_rows[PR:, tt] = R
                    nrows[tt] = PR
            else:
                for a in range(take):
                    tt = t0 + t + a
                    rows = r0 + (t + a) * 128 + np.arange(128)
                    local_rows[:, tt] = np.minimum(rows, SHARD_ROWS - 1)
                    nrows[tt] = 128
            t += take
        for a in range(L["n"]):
            lane_of[t0 + a] = li
            idx_in[t0 + a] = a
        groups.append(gl)
        t0 += L["n"]
        r0 += L["n"] * (PR if L["kind"] == "plain" else 128)
    # ACT-Relu tiles: every ACT-lane tile (so its bins are engine-local,
    # interleaving with its own dma stream and never stalling DVE), plus a
    # few mid-late gather tiles once ACT's dma stream has drained.
    eng = np.zeros(NTILES, np.int64)  # 0 = DVE is_gt, 1 = ACT relu
    eng[lane_of == 1] = 1
    extra = N_RELU - int((lane_of == 1).sum())
    gt_ids = np.nonzero(lane_of == 2)[0]
    for i in range(max(0, extra)):
        eng[gt_ids[-2 - i]] = 1
    return lane_of, idx_in, nrows, local_rows, groups, eng


LANE_OF, IDX_IN, NROWS, LOCAL_ROWS, GROUPS, ENG_OF = _build_tables()

# interleaved emission order: sort groups by estimated arrival time
_LANE_TILE_NS = (1974, 1974, 1130)
_LANE_HEAD_NS = (2600, 2600, 2500)


def _schedule():
    evs = []
    for li, gl in enumerate(GROUPS):
        acc = _LANE_HEAD_NS[li]
        for gi, (tf, T) in enumerate(gl):
            acc += T * _LANE_TILE_NS[li]
            evs.append((acc, li, gi))
    evs.sort()
    return evs


SCHEDULE = _schedule()
GIDX_COLS = -(-(128 * N_GT) // 16)


def build_program():
    nc = bacc.Bacc(
        "TRN2", target_bir_lowering=False, debug=False, enable_asserts=False
    )
    masks = nc.dram_tensor("masks", [SHARD_ROWS, W], fp32, kind="ExternalInput").ap()
    sel = nc.dram_tensor("sel", [128, NTILES * GSZ], fp8, kind="ExternalInput").ap()
    gidx = nc.dram_tensor("gidx", [128, GIDX_COLS], i16, kind="ExternalInput").ap()
    rowany_out = [
        nc.dram_tensor(f"rowany_{L['name']}", [128, L["n"]], fp32,
                       kind="ExternalOutput").ap()
        for L in LANES
    ]
    counts_out = [
        nc.dram_tensor(f"counts_{L['name']}", [128, NCHUNK * GSZ], fp16,
                       kind="ExternalOutput").ap()
        for L in LANES
    ]

    qeng = ("sync", "scalar", "gpsimd")

    with tile.TileContext(nc) as tc:
        with (
            tc.tile_pool(name="raw", bufs=3) as rawp,
            tc.tile_pool(name="gout", bufs=4) as goutp,
            tc.tile_pool(name="bins", bufs=6) as binp,
            tc.tile_pool(name="binh", bufs=4) as binhp,
            tc.tile_pool(name="consts", bufs=1) as constp,
            tc.tile_pool(name="psum", bufs=1, space="PSUM") as psump,
        ):
            sel_t = constp.tile([128, NTILES * GSZ], fp8)
            gidx_t = constp.tile([128, GIDX_COLS], i16)
            bias = constp.tile([128, 1], fp32)
            scr = constp.tile([128, 1], fp32)
            nc.gpsimd.memset(bias[:], -THRESHOLD)
            rowany_t = [
                constp.tile([128, L["n"]], fp32, name=f"ra{L['name']}")
                for L in LANES
            ]
            cc = [
                psump.tile([128, NCHUNK * GSZ], fp32, name=f"cc{li}",
                           tag=f"cc{li}")
                for li in range(len(LANES))
            ]
            gsem = nc.alloc_semaphore("gdma")

            raw_tiles = {}
            gather_count = [0]
            gather_of_tile = {}

            def emit_dma(li, gi):
                L = LANES[li]
                tf, T = GROUPS[li][gi]
                if L["kind"] == "plain":
                    R0 = int(LOCAL_ROWS[0, tf])
                    raw = rawp.tile([128, L["g"] * W], fp32,
                                    name=f"raw{li}", tag=f"raw{li}")
                    getattr(nc, L["eng"]).dma_start(
                        raw[:PR, : T * W],
                        masks[R0 : R0 + T * PR, :].rearrange(
                            "(p a) x -> p (a x)", p=PR),
                    )
                else:
                    nid = 128 * T
                    c0 = (tf - N_PLAIN) * 8
                    raw = goutp.tile([128, K_GT * W], fp32, name="gout",
                                     tag="gout")
                    nc.gpsimd.dma_gather(
                        raw[:, : T * W].rearrange("p (k e) -> p k e", e=W),
                        masks,
                        gidx_t[:16, c0 : c0 + nid // 16],
                        nid, nid, W,
                        prepare_only=True, sem=gsem,
                    )
                    nc.gpsimd.trigger_dma(count=None)
                    gather_count[0] += 1
                    for a in range(T):
                        gather_of_tile[tf + a] = gather_count[0]
                raw_tiles[(li, gi)] = raw

            waited = {"v": 0, "a": 0}

            def emit_compute(li, gi):
                L = LANES[li]
                tf, T = GROUPS[li][gi]
                raw = raw_tiles.pop((li, gi))
                for a in range(T):
                    t = tf + a
                    P = int(NROWS[t])
                    ra = rowany_t[li][:P, IDX_IN[t] : IDX_IN[t] + 1]
                    if L["kind"] == "gather":
                        need = 16 * gather_of_tile[t]
                        key = "v" if ENG_OF[t] == 0 else "a"
                        if waited[key] < need:
                            eng = nc.vector if key == "v" else nc.scalar
                            eng.wait_ge(gsem, need)
                            waited[key] = need
                    if ENG_OF[t] == 0:
                        b = binp.tile([128, W], fp8, name="b", tag="b")
                        nc.vector.tensor_scalar(
                            out=b[:P, :], in0=raw[:P, a * W : (a + 1) * W],
                            scalar1=THRESHOLD, scalar2=None, op0=Op.is_gt,
                            op1=Op.max, accum_out=ra,
                        )
                    else:
                        b = binhp.tile([128, W], fp16, name="bh", tag="bh")
                        nc.scalar.activation(
                            out=b[:P, :], in_=raw[:P, a * W : (a + 1) * W],
                            func=AF.Relu, bias=bias[:P, :1], accum_out=ra,
                        )
                    first, last = LANES[li]["_tf"], LANES[li]["_tl"]
                    for ci in range(NCHUNK):
                        nc.tensor.matmul(
                            cc[li][:, ci * GSZ : (ci + 1) * GSZ],
                            b[:P, ci * 128 : (ci + 1) * 128],
                            sel_t[:P, t * GSZ : t * GSZ + GSZ],
                            start=(t == first), stop=(t == last),
                        )

            for li, L in enumerate(LANES):
                tids = np.nonzero(LANE_OF == li)[0]
                L["_tf"], L["_tl"] = int(tids[0]), int(tids[-1])

            # head: idxs on ACT (small first group), warmup, sel on SP
            # right behind its first 1-tile data dma
            nc.scalar.dma_start(gidx_t[:], gidx)
            nc.scalar.activation(out=scr[:1, :1], in_=bias[:1, :1],
                                 func=AF.Relu, bias=bias[:1, :1])

            def emit_outputs(li):
                # drain: DVE copy PSUM -> fp16 stage, out on the lane's queue
                stage = constp.tile([128, NCHUNK * GSZ], fp16,
                                    name=f"st{li}", tag=f"st{li}")
                nc.vector.tensor_scalar(out=stage[:], in0=cc[li][:, :],
                                        scalar1=0.0, scalar2=None, op0=Op.add)
                getattr(nc, qeng[li]).dma_start(counts_out[li], stage[:])
                getattr(nc, qeng[li]).dma_start(rowany_out[li], rowany_t[li][:])

            for li in range(len(LANES)):
                if GROUPS[li]:
                    emit_dma(li, 0)
            nc.sync.dma_start(sel_t[:], sel)
            # early lanes' drains are emitted a few schedule events after the
            # lane's last compute: far enough that in-order DVE never blocks
            # on the drain's stop-matmul wait, early enough to leave the end
            # chain to the final lane only
            pending = []
            for idx, (_, li, gi) in enumerate(SCHEDULE):
                if gi + 1 < len(GROUPS[li]):
                    emit_dma(li, gi + 1)
                emit_compute(li, gi)
                if gi == len(GROUPS[li]) - 1 and li != 2:
                    pending.append([idx + 3, li])
                for ent in list(pending):
                    if idx >= ent[0]:
                        emit_outputs(ent[1])
                        pending.remove(ent)
            for ent in pending:
                emit_outputs(ent[1])
            emit_outputs(2)

    nc.compile()
    return nc


def make_sel(core):
    """Per-tile one-hot selector: partition p -> local mask index."""
    g = core * SHARD_ROWS + LOCAL_ROWS  # [128, NTILES] global rows
    first = (core * SHARD_ROWS) // H
    ul = g // H - first
    assert ul.min() >= 0 and ul.max() < GSZ
    sel = np.zeros((128, NTILES * GSZ), ml_dtypes.float8_e4m3)
    for t in range(NTILES):
        P = int(NROWS[t])
        sel[np.arange(P), t * GSZ + ul[:P, t]] = 1
    return sel


def make_gidx():
    """Gather row indices, 16-partition wrap replicated across 128."""
    idxs = LOCAL_ROWS[:, N_PLAIN:]  # [128, N_GT]
    flat = idxs.T.reshape(-1)
    base = np.zeros((16, GIDX_COLS), np.int16)
    for j, v in enumerate(flat):
        base[j % 16, j // 16] = v
    return np.tile(base, (8, 1))


def postprocess(results):
    """Per-core rowany/counts -> boxes [N, 2, 2] f32 (exact)."""
    v1 = np.zeros(N)  # H - ymin   (0 if empty)
    v2 = np.zeros(N)  # ymax + 1
    u1 = np.zeros(N)  # W - xmin
    u2 = np.zeros(N)  # xmax + 1
    xs = np.arange(W)
    for c, r in enumerate(results):
        g = c * SHARD_ROWS + LOCAL_ROWS
        unit = g // H
        y = g % H
        first = (c * SHARD_ROWS) // H
        nu = ((c + 1) * SHARD_ROWS - 1) // H - first + 1
        colany = np.zeros((nu, W), bool)
        for li, L in enumerate(LANES):
            tids = np.nonzero(LANE_OF == li)[0]
            P = int(NROWS[tids[0]])
            ra = np.asarray(r[f"rowany_{L['name']}"])[:P]  # [P, n]
            any_ = ra > 0
            ub, yb = unit[:P, tids], y[:P, tids]
            np.maximum.at(v1, ub[any_], (H - yb)[any_])
            np.maximum.at(v2, ub[any_], (yb + 1)[any_])
            cnt = np.asarray(r[f"counts_{L['name']}"]).astype(np.float64)
            cols = cnt.reshape(128, NCHUNK, GSZ).transpose(2, 1, 0).reshape(GSZ, W)
            colany[:nu] |= (cols > 0)[:nu]
        np.maximum.at(u1, first + np.arange(nu),
                      np.max(np.where(colany, W - xs, 0), 1))
        np.maximum.at(u2, first + np.arange(nu),
                      np.max(np.where(colany, xs + 1, 0), 1))
    boxes = np.empty((N, 2, 2), np.float32)
    boxes[:, 0, 0] = W - u1  # xmin
    boxes[:, 0, 1] = H - v1  # ymin
    boxes[:, 1, 0] = u2 - 1  # xmax
    boxes[:, 1, 1] = v2 - 1  # ymax
    return boxes


_cache = {}


def _get_program():
    if "nc" not in _cache:
        _cache["nc"] = build_program()
        _cache["sel"] = [make_sel(c) for c in range(N_CORES)]
        _cache["gidx"] = make_gidx()
    return _cache["nc"], _cache["sel"]


def make_in_maps(masks):
    masks = np.ascontiguousarray(np.asarray(masks, dtype=np.float32))
    _, sels = _get_program()
    gidx = _cache["gidx"]
    flat = masks.reshape(ROWS, W)
    return [
        {"masks": flat[c * SHARD_ROWS : (c + 1) * SHARD_ROWS],
         "sel": sels[c], "gidx": gidx}
        for c in range(N_CORES)
    ]


def kernel(masks):
    nc, _ = _get_program()
    in_maps = make_in_maps(masks)
    res = run_bass_kernel_spmd(nc, in_maps, core_ids=list(range(N_CORES)))
    return postprocess(res.results)
